# revision 81
# baseline (speedup 1.0000x reference)
"""HOPE block kernel for 8 Trainium2 NeuronCores.

Sharding: attention is head-parallel (8 heads -> 8 cores, each core runs the
full-sequence linear-attention scan for its head locally), everything
token-parallel elsewhere (layernorms, output projection, CMS MLPs on a
1024-token shard per core).  Cross-core movement: one AllGather of the
ln1-normalized activations (transposed, bf16) + one AllToAll of the
per-head attention outputs.

M = cumsum_t(mean_b V K^T) readout is computed with a chunked scan:
  Y^T_chunk = G^T Q^T + V^T (mask o K Q^T),  G += sum_b V_b^T K_b
with the 1/B scale and the ln scale/bias folded into the projection weights
host-side.
"""

import numpy as np
import ml_dtypes

import concourse.bass as bass
import concourse.bacc as bacc
import concourse.mybir as mybir
import concourse.tile as tile
from concourse.bass_utils import run_bass_kernel_spmd
from concourse.masks import make_identity

LAST_RESULT = None  # set to the BassKernelResults of the most recent run

N_CORES = 8
B, S, DIM = 4, 2048, 512
H, D = 8, 64
HID = 4 * DIM
NLVL = 3
EPS = 1e-5
TOK = B * S              # 8192 flat tokens
TSH = TOK // N_CORES     # 1024 tokens per shard
NT = TSH // 128          # 8 token tiles per shard
NCH = S // 128           # 16 sequence chunks per batch
FP32 = mybir.dt.float32
BF16 = mybir.dt.bfloat16
AX = mybir.AxisListType.X
ALU = mybir.AluOpType
ACTF = mybir.ActivationFunctionType


def _ln_normalize(nc, pool, xt, out_bf, sq_scratch, eps_tile):
    """out_bf = (xt - mean(xt)) * rsqrt(var(xt)+EPS), per 128-token tile."""
    ssum = pool.tile([128, 1], FP32, tag="ln_s")
    sumsq = pool.tile([128, 1], FP32, tag="ln_q")
    nc.vector.tensor_reduce(ssum[:], xt[:], AX, ALU.add)
    nc.scalar.activation(sq_scratch[:], xt[:], ACTF.Square, accum_out=sumsq[:])
    negmu = pool.tile([128, 1], FP32, tag="ln_m")
    nc.vector.tensor_scalar_mul(negmu[:], ssum[:], -1.0 / DIM)
    e2 = pool.tile([128, 1], FP32, tag="ln_e")
    nc.vector.tensor_scalar_mul(e2[:], sumsq[:], 1.0 / DIM)
    mu2 = pool.tile([128, 1], FP32, tag="ln_2")
    nc.vector.tensor_tensor(mu2[:], negmu[:], negmu[:], ALU.mult)
    var = pool.tile([128, 1], FP32, tag="ln_v")
    nc.vector.tensor_tensor(var[:], e2[:], mu2[:], ALU.subtract)
    std = pool.tile([128, 1], FP32, tag="ln_d")
    nc.scalar.activation(std[:], var[:], ACTF.Sqrt, bias=eps_tile[:])
    rs = pool.tile([128, 1], FP32, tag="ln_r")
    nc.vector.reciprocal(rs[:], std[:])
    nc.vector.tensor_scalar(
        out=out_bf[:], in0=xt[:], scalar1=negmu[:], scalar2=rs[:],
        op0=ALU.add, op1=ALU.mult,
    )


def build_kernel():
    nc = bacc.Bacc(num_devices=N_CORES)

    x_sh = nc.dram_tensor("x_shard", [TSH, DIM], FP32, kind="ExternalInput")
    qk_wT = nc.dram_tensor("qk_wT", [DIM, 128], BF16, kind="ExternalInput")
    v_wT = nc.dram_tensor("v_wT", [DIM, D], BF16, kind="ExternalInput")
    qkv_b = nc.dram_tensor("qkv_b", [3, D, 1], FP32, kind="ExternalInput")
    wo_T = nc.dram_tensor("wo_T", [DIM, DIM], BF16, kind="ExternalInput")
    w1 = nc.dram_tensor("w1", [NLVL, DIM, HID], BF16, kind="ExternalInput")
    w2 = nc.dram_tensor("w2", [NLVL, HID, DIM], BF16, kind="ExternalInput")
    b1 = nc.dram_tensor("b1", [NLVL, 128, HID // 128], FP32, kind="ExternalInput")
    b2a = nc.dram_tensor("b2a", [2, 128, DIM // 128], FP32, kind="ExternalInput")
    b2last = nc.dram_tensor("b2last", [128, DIM], FP32, kind="ExternalInput")
    out_sh = nc.dram_tensor("out_shard", [TSH, DIM], FP32, kind="ExternalOutput")

    with tile.TileContext(nc) as tc:
        with tc.tile_pool(name="dram", bufs=1, space="DRAM") as dram, \
             tc.tile_pool(name="const", bufs=1) as cpool, \
             tc.tile_pool(name="lns", bufs=4) as lnp, \
             tc.tile_pool(name="xp", bufs=1) as xpool:

            ag_in = dram.tile([DIM, TSH], BF16)
            ag_out = dram.tile([N_CORES * DIM, TSH], BF16)
            a2a_in = dram.tile([N_CORES * D, TSH], BF16)
            a2a_out = dram.tile([N_CORES * D, TSH], BF16)

            identity = cpool.tile([128, 128], BF16)
            make_identity(nc, identity[:])
            identity64 = cpool.tile([D, D], BF16)
            make_identity(nc, identity64[:])
            # keep-mask: mask[t, s] = 1.0 if t <= s else 0.0
            mask = cpool.tile([128, 128], FP32)
            nc.gpsimd.memset(mask[:], 1.0)
            nc.gpsimd.affine_select(
                out=mask[:], in_=mask[:], compare_op=ALU.is_ge, fill=0.0,
                base=0, pattern=[[1, 128]], channel_multiplier=-1,
            )

            qkw_sb = cpool.tile([128, 4, 128], BF16)
            nc.sync.dma_start(qkw_sb[:], qk_wT[:].rearrange("(a p) m -> p a m", p=128))
            vw_sb = cpool.tile([128, 4, D], BF16)
            nc.sync.dma_start(vw_sb[:], v_wT[:].rearrange("(a p) m -> p a m", p=128))
            qb_sb = cpool.tile([D, 3], FP32)
            nc.sync.dma_start(qb_sb[:], qkv_b[:].rearrange("c p one -> p (c one)"))
            woT_sb = cpool.tile([128, 4, DIM], BF16)
            nc.sync.dma_start(woT_sb[:], wo_T[:].rearrange("(a p) m -> p a m", p=128))
            b2l_sb = cpool.tile([128, DIM], FP32)
            nc.sync.dma_start(b2l_sb[:], b2last[:])
            eps_sb = cpool.tile([128, 1], FP32)
            nc.vector.memset(eps_sb[:], EPS)

            x_sb = xpool.tile([128, NT, DIM], FP32)
            nc.sync.dma_start(x_sb[:], x_sh[:].rearrange("(t p) d -> p t d", p=128))

            # ---- stage 1: ln1 on own token shard, transpose, AllGather ----
            with nc.named_scope("s1_ln1"), \
                 tc.tile_pool(name="hT", bufs=1) as hTp, \
                 tc.tile_pool(name="s1w", bufs=3) as s1w, \
                 tc.tile_pool(name="s1p", bufs=2, space="PSUM") as s1p:
                hT_sb = hTp.tile([128, 4, TSH], BF16)
                for t in range(NT):
                    hn = s1w.tile([128, DIM], BF16, tag="hn")
                    sq = s1w.tile([128, DIM], BF16, tag="sq")
                    _ln_normalize(nc, lnp, x_sb[:, t], hn, sq, eps_sb)
                    for a in range(4):
                        ps = s1p.tile([128, 128], BF16)
                        nc.tensor.transpose(ps[:], hn[:, a * 128:(a + 1) * 128], identity[:])
                        nc.vector.tensor_copy(hT_sb[:, a, t * 128:(t + 1) * 128], ps[:])
                nc.sync.dma_start(ag_in[:].rearrange("(a p) n -> p a n", p=128), hT_sb[:])

            with nc.named_scope("c_allgather"):
                nc.gpsimd.collective_compute(
                    "AllGather", ALU.bypass,
                    replica_groups=[list(range(N_CORES))],
                    ins=[ag_in.opt()], outs=[ag_out.opt()],
                )
            agv = ag_out[:].rearrange("(s a p) n -> s a p n", s=N_CORES, a=4)

            # ---- stage 2+3: per-head QKV projections + chunked scan ----
            with tc.tile_pool(name="qkv", bufs=1) as qkvp, \
                 tc.tile_pool(name="s2w", bufs=4) as s2w:
                s2ctx = tc.tile_pool(name="s2p", bufs=2, space="PSUM")
                s2p = s2ctx.__enter__()
                s2vctx = tc.tile_pool(name="s2pv", bufs=2, space="PSUM")
                s2pv = s2vctx.__enter__()
                s2tctx = tc.tile_pool(name="s2pt", bufs=2, space="PSUM")
                s2pt = s2tctx.__enter__()
                QT = qkvp.tile([D, TOK], BF16)
                KT = qkvp.tile([D, TOK], BF16)
                VT = qkvp.tile([D, TOK], BF16)
                K_td = qkvp.tile([128, 64 * D], BF16)
                V_td = qkvp.tile([128, 64 * D], BF16)

                s2scope = nc.named_scope("s2_qkv")
                s2scope.__enter__()
                for blk in range(16):          # 512-token blocks
                    s, h2 = blk // 2, blk % 2
                    col = slice(blk * 512, (blk + 1) * 512)
                    rhs = []
                    for a in range(4):
                        r = s2w.tile([128, 512], BF16, tag="rhs")
                        nc.sync.dma_start(r[:], agv[s, a, :, h2 * 512:(h2 + 1) * 512])
                        rhs.append(r)
                    pqk = s2p.tile([128, 512], FP32)
                    for a in range(4):
                        nc.tensor.matmul(pqk[:], qkw_sb[:, a], rhs[a][:],
                                         start=(a == 0), stop=(a == 3))
                    pv = s2pv.tile([D, 512], FP32)
                    for a in range(4):
                        nc.tensor.matmul(pv[:], vw_sb[:, a], rhs[a][:],
                                         start=(a == 0), stop=(a == 3))
                    nc.scalar.activation(QT[:, col], pqk[0:D, :], ACTF.Identity,
                                         bias=qb_sb[:, 0:1])
                    nc.scalar.activation(KT[:, col], pqk[D:128, :], ACTF.Identity,
                                         bias=qb_sb[:, 1:2])
                    nc.scalar.activation(VT[:, col], pv[:], ACTF.Identity,
                                         bias=qb_sb[:, 2:3])
                    for u in range(4):         # transpose K,V 128-tok subtiles
                        tt = blk * 4 + u
                        csub = slice(tt * 128, (tt + 1) * 128)
                        pk = s2pt.tile([128, D], BF16, tag="pk")
                        nc.tensor.transpose(pk[:], KT[:, csub], identity64[:])
                        nc.vector.tensor_copy(K_td[:, tt * D:(tt + 1) * D], pk[:])
                        pvv = s2pt.tile([128, D], BF16, tag="pv")
                        nc.tensor.transpose(pvv[:], VT[:, csub], identity64[:])
                        nc.vector.tensor_copy(V_td[:, tt * D:(tt + 1) * D], pvv[:])

                s2tctx.__exit__(None, None, None)
                s2vctx.__exit__(None, None, None)
                s2ctx.__exit__(None, None, None)
                s2scope.__exit__(None, None, None)
                # ---- chunked scan ----
                G32 = qkvp.tile([D, D], FP32)
                G16 = qkvp.tile([D, D], BF16)
                nc.vector.memset(G32[:], 0.0)
                nc.vector.memset(G16[:], 0.0)

                with nc.named_scope("s3_scan"), \
                     tc.tile_pool(name="tm", bufs=18) as tmp_pool, \
                     tc.tile_pool(name="yev", bufs=4) as yev, \
                     tc.tile_pool(name="pt3", bufs=3, space="PSUM") as pt3, \
                     tc.tile_pool(name="py3", bufs=2, space="PSUM") as py3, \
                     tc.tile_pool(name="pg3", bufs=1, space="PSUM") as pg3:
                    for sc in range(NCH):
                        ttb = [bb * NCH + sc for bb in range(B)]
                        Tm = {}
                        for bp in range(B):
                            kcol = slice(ttb[bp] * 128, ttb[bp] * 128 + 128)
                            for b in range(B):
                                qcol = slice(ttb[b] * 128, ttb[b] * 128 + 128)
                                pt = pt3.tile([128, 128], FP32)
                                nc.tensor.matmul(pt[:], KT[:, kcol], QT[:, qcol])
                                tm = tmp_pool.tile([128, 128], BF16)
                                nc.vector.tensor_tensor(tm[:], pt[:], mask[:], ALU.mult)
                                Tm[(bp, b)] = tm
                        for b in range(B):
                            qcol = slice(ttb[b] * 128, ttb[b] * 128 + 128)
                            py = py3.tile([D, 128], FP32)
                            nc.tensor.matmul(py[:], G16[:], QT[:, qcol],
                                             start=True, stop=False)
                            for bp in range(B):
                                vcol = slice(ttb[bp] * D, (ttb[bp] + 1) * D)
                                nc.tensor.matmul(py[:], V_td[:, vcol], Tm[(bp, b)][:],
                                                 start=False, stop=(bp == B - 1))
                            ye = yev.tile([D, 128], BF16)
                            nc.scalar.activation(ye[:], py[:], ACTF.Copy)
                            flat = b * S + sc * 128
                            j, off = flat // TSH, flat % TSH
                            nc.sync.dma_start(
                                a2a_in[j * D:(j + 1) * D, off:off + 128], ye[:])
                        pg = pg3.tile([D, D], FP32)
                        for b in range(B):
                            vcol = slice(ttb[b] * D, (ttb[b] + 1) * D)
                            nc.tensor.matmul(pg[:], K_td[:, vcol], V_td[:, vcol],
                                             start=(b == 0), stop=(b == B - 1))
                        nc.vector.tensor_tensor(G32[:], G32[:], pg[:], ALU.add)
                        nc.vector.tensor_copy(G16[:], G32[:])

            with nc.named_scope("c_alltoall"):
                nc.gpsimd.collective_compute(
                    "AllToAll", ALU.bypass,
                    replica_groups=[list(range(N_CORES))],
                    ins=[a2a_in.opt()], outs=[a2a_out.opt()],
                )

            # ---- stage 4: Wo + residual + ln2 + transpose ----
            h2nT_p = tc.tile_pool(name="h2nT", bufs=1)
            h2_p = tc.tile_pool(name="h2", bufs=1)
            h2nT = h2nT_p.__enter__().tile([128, 4, TSH], BF16)
            h2_sb = h2_p.__enter__().tile([128, NT, DIM], FP32)
            with nc.named_scope("s4_wo_ln2"), \
                 tc.tile_pool(name="s4w", bufs=4) as s4w, \
                 tc.tile_pool(name="s4p", bufs=2, space="PSUM") as s4p, \
                 tc.tile_pool(name="s4pt", bufs=2, space="PSUM") as s4pt:
                for t in range(NT):
                    po = s4p.tile([128, DIM], FP32)
                    for dk in range(4):
                        lh = s4w.tile([128, 128], BF16, tag="yT")
                        nc.sync.dma_start(
                            lh[:], a2a_out[dk * 128:(dk + 1) * 128, t * 128:(t + 1) * 128])
                        nc.tensor.matmul(po[:], lh[:], woT_sb[:, dk],
                                         start=(dk == 0), stop=(dk == 3))
                    nc.vector.tensor_tensor(h2_sb[:, t], po[:], x_sb[:, t], ALU.add)
                    hn = s4w.tile([128, DIM], BF16, tag="hn2")
                    sq = s4w.tile([128, DIM], BF16, tag="sq2")
                    _ln_normalize(nc, lnp, h2_sb[:, t], hn, sq, eps_sb)
                    for a in range(4):
                        ps = s4pt.tile([128, 128], BF16)
                        nc.tensor.transpose(ps[:], hn[:, a * 128:(a + 1) * 128], identity[:])
                        nc.vector.tensor_copy(h2nT[:, a, t * 128:(t + 1) * 128], ps[:])

            # ---- stage 5: CMS chain ----
            with nc.named_scope("s5_cms"), \
                 tc.tile_pool(name="g", bufs=1) as gp, \
                 tc.tile_pool(name="wts", bufs=2) as wts, \
                 tc.tile_pool(name="bts", bufs=2) as bts, \
                 tc.tile_pool(name="s5o", bufs=3) as s5o, \
                 tc.tile_pool(name="s5p", bufs=4, space="PSUM") as s5p:
                g_sb = gp.tile([128, 16, TSH], BF16)
                cur = h2nT
                for lvl in range(NLVL):
                    w1_sb = wts.tile([128, 4, 16, 128], BF16, tag="w")
                    nc.sync.dma_start(
                        w1_sb[:],
                        w1[lvl].rearrange("(a p) (ht q) -> p a ht q", p=128, q=128))
                    b1_sb = bts.tile([128, HID // 128], FP32, tag="b1")
                    nc.sync.dma_start(b1_sb[:], b1[lvl])
                    for ht in range(16):
                        for nh in range(2):
                            colw = slice(nh * 512, nh * 512 + 512)
                            ps = s5p.tile([128, 512], FP32)
                            for a in range(4):
                                nc.tensor.matmul(ps[:], w1_sb[:, a, ht],
                                                 cur[:, a, colw],
                                                 start=(a == 0), stop=(a == 3))
                            nc.scalar.activation(
                                g_sb[:, ht, colw], ps[:], ACTF.Gelu_apprx_tanh,
                                bias=b1_sb[:, ht:ht + 1])
                    w2_sb = wts.tile([128, 16, 4, 128], BF16, tag="w")
                    nc.sync.dma_start(
                        w2_sb[:],
                        w2[lvl].rearrange("(ht p) (a q) -> p ht a q", p=128, q=128))
                    if lvl < 2:
                        b2_sb = bts.tile([128, 4], FP32, tag="b2")
                        nc.sync.dma_start(b2_sb[:], b2a[lvl])
                        nxt = s5o.tile([128, 4, TSH], BF16, tag="nxt")
                        for a in range(4):
                            for nh in range(2):
                                colw = slice(nh * 512, nh * 512 + 512)
                                ps = s5p.tile([128, 512], FP32)
                                for ht in range(16):
                                    nc.tensor.matmul(ps[:], w2_sb[:, ht, a],
                                                     g_sb[:, ht, colw],
                                                     start=(ht == 0), stop=(ht == 15))
                                nc.scalar.activation(
                                    nxt[:, a, colw], ps[:], ACTF.Identity,
                                    bias=b2_sb[:, a:a + 1])
                        cur = nxt
                    else:
                        # last level: emit [tok, f], add b2 + residual, write out
                        w2r_sb = wts.tile([128, 16, 512], BF16, tag="w2r")
                        nc.sync.dma_start(
                            w2r_sb[:],
                            w2[2].rearrange("(ht p) m -> p ht m", p=128))
                        for t in range(NT):
                            ps = s5p.tile([128, 512], FP32)
                            for ht in range(16):
                                nc.tensor.matmul(
                                    ps[:], g_sb[:, ht, t * 128:(t + 1) * 128],
                                    w2r_sb[:, ht],
                                    start=(ht == 0), stop=(ht == 15))
                            tmp = s5o.tile([128, DIM], FP32, tag="fin")
                            nc.vector.tensor_tensor(tmp[:], ps[:], b2l_sb[:], ALU.add)
                            nc.vector.tensor_tensor(tmp[:], tmp[:], h2_sb[:, t], ALU.add)
                            nc.sync.dma_start(
                                out_sh[:].rearrange("(t p) d -> p t d", p=128)[:, t],
                                tmp[:])
            h2_p.__exit__(None, None, None)
            h2nT_p.__exit__(None, None, None)

    nc.finalize()
    return nc


_NC_CACHE = {}


def _get_nc():
    if "nc" not in _NC_CACHE:
        _NC_CACHE["nc"] = build_kernel()
    return _NC_CACHE["nc"]


def kernel(x, Wq, Wk, Wv, Wo, ln1_w, ln1_b, ln2_w, ln2_b,
           cms_W1, cms_b1, cms_W2, cms_b2):
    bf = ml_dtypes.bfloat16
    f32 = np.float32
    x = np.asarray(x, f32)
    ln1_w = np.asarray(ln1_w, f32); ln1_b = np.asarray(ln1_b, f32)
    ln2_w = np.asarray(ln2_w, f32); ln2_b = np.asarray(ln2_b, f32)

    # fold ln1 scale into Wq/Wk/Wv columns, ln1 bias into additive biases
    Wq = np.asarray(Wq, f32); Wk = np.asarray(Wk, f32); Wv = np.asarray(Wv, f32)
    Wo = np.asarray(Wo, f32)
    Wqs = Wq * ln1_w[None, :]; Wks = Wk * ln1_w[None, :]
    Wvs = (Wv * ln1_w[None, :]) / B
    bq = Wq @ ln1_b; bk = Wk @ ln1_b; bv = (Wv @ ln1_b) / B

    W1 = np.asarray(cms_W1, f32).copy(); b1v = np.asarray(cms_b1, f32).copy()
    W2 = np.asarray(cms_W2, f32); b2v = np.asarray(cms_b2, f32)
    b1v[0] = b1v[0] + ln2_b @ W1[0]
    W1[0] = W1[0] * ln2_w[:, None]

    xf = x.reshape(TOK, DIM)
    b1r = np.ascontiguousarray(
        b1v.reshape(NLVL, HID // 128, 128).transpose(0, 2, 1))
    b2ar = np.ascontiguousarray(
        b2v[:2].reshape(2, DIM // 128, 128).transpose(0, 2, 1))
    b2last = np.broadcast_to(b2v[2], (128, DIM)).copy()

    in_maps = []
    for c in range(N_CORES):
        hs = slice(c * D, (c + 1) * D)
        qk_wT = np.concatenate([Wqs[hs].T, Wks[hs].T], axis=1)  # [512, 128]
        in_maps.append({
            "x_shard": np.ascontiguousarray(xf[c * TSH:(c + 1) * TSH]),
            "qk_wT": qk_wT.astype(bf),
            "v_wT": np.ascontiguousarray(Wvs[hs].T).astype(bf),
            "qkv_b": np.stack([bq[hs], bk[hs], bv[hs]])[:, :, None].astype(f32),
            "wo_T": np.ascontiguousarray(Wo.T).astype(bf),
            "w1": W1.astype(bf),
            "w2": W2.astype(bf),
            "b1": b1r.astype(f32),
            "b2a": b2ar.astype(f32),
            "b2last": b2last.astype(f32),
        })

    nc = _get_nc()
    res = run_bass_kernel_spmd(nc, in_maps, core_ids=list(range(N_CORES)))
    global LAST_RESULT
    LAST_RESULT = res
    out = np.concatenate([res.results[c]["out_shard"] for c in range(N_CORES)], axis=0)
    return out.reshape(B, S, DIM).astype(np.float32)

